# revision 4
# baseline (speedup 1.0000x reference)
"""Frequency-Channel-Attention kernel for Trainium2 (8 NeuronCores, SPMD), v4.

Math: dct2(X) = D @ X @ D^T with D[k,j] = cos(pi*k*(2j+1)/(2L))/L, L=64.
Per (b,c): S = max(dct2(ip[b,c])); h = relu(S@w1); z = sigmoid(h@w2);
out = ip * z[b,c].

v4 = v3 dataflow + software-pipelined emission (1-j lookahead keeps the
PE dense/warm) + engine reassignments.

Per-j (j = 2*b + cg, 128 channels) dataflow:
  - SWDGE casting loads: T16[j][cl, (h,w)] bf16, h = 2H + r.
  - T1 (PE): 32 chunk transposes -> psa bf16 -> ACT evac ->
    X[(r,w), (H, cl)].
  - s1 (PE): lhsT = BD1[(r,w),(r',k2)] = d_rr' D[k2,w]; rhs streamed
    (cl slow, H fast) per 32-channel quarter -> ps1[(r',k2), (cl, H)] f32.
  - ACT cast evac ps1 -> A bf16; DVE StreamTranspose (32x32 blocks)
    A -> R[(r',k2h,H), (cl, k2m)].
  - s2 (PE, single-shot): lhsT = BD2p[(r',k2h,H),(k2h',k1)] =
    d_k2h D[k1, 2H+r'] -> ps2[(k2h,k1), (cl, k2m)] f32.
  - DVE reduce_max over k2m -> Mx[j][(k2h,k1), cl]; per-j PE transpose
    of Mx + DVE reduce -> S[cl] per j.
  - MLP on PE/ACT per batch; multiply T16*z (gp/DVE/ACT) -> f32;
    HWDGE f32 stores on sync.
"""

import os
import sys

import numpy as np

for _p in ("/opt/trn_rl_repo", "/opt/pypackages"):
    if os.path.isdir(_p) and _p not in sys.path:
        sys.path.append(_p)

import concourse.bacc as bacc
import concourse.tile as tile
from concourse import mybir
from concourse.bass_utils import run_bass_kernel_spmd

F32 = mybir.dt.float32
BF16 = mybir.dt.bfloat16

B, C, H, W = 16, 256, 64, 64
N_CORES = 8
B_LOC = B // N_CORES
NJ = 4  # j = 2*b + cg

_NC_CACHE = {}


def _dct_matrix():
    k = np.arange(W, dtype=np.float64)[:, None]
    j = np.arange(W, dtype=np.float64)[None, :]
    D = np.cos(np.pi * k * (2.0 * j + 1.0) / (2.0 * W)) / W
    return D.astype(np.float32)


def _constants():
    D = _dct_matrix()
    BD1 = np.zeros((128, 128), dtype=np.float32)  # [(r,w), (r',k2)]
    for r in range(2):
        BD1[64 * r : 64 * r + 64, 64 * r : 64 * r + 64] = D.T
    BD2p = np.zeros((128, 128), dtype=np.float32)  # [(r',k2h,hb), (k2h',k1)]
    for rp in range(2):
        for k2h in range(2):
            base = 32 * (2 * rp + k2h)
            BD2p[base : base + 32, 64 * k2h : 64 * k2h + 64] = D.T[rp::2, :]
    identf = np.eye(128, dtype=np.float32)
    return BD1, BD2p, identf


def _build_nc():
    nc = bacc.Bacc(None, target_bir_lowering=False)
    ip_d = nc.dram_tensor("ip", [B_LOC, C, H, W], F32, kind="ExternalInput")
    w1a_d = nc.dram_tensor("w1a", [128, 16], F32, kind="ExternalInput")
    w1b_d = nc.dram_tensor("w1b", [128, 16], F32, kind="ExternalInput")
    w2_d = nc.dram_tensor("w2", [16, C], F32, kind="ExternalInput")
    bd1_d = nc.dram_tensor("bd1", [128, 128], F32, kind="ExternalInput")
    bd2_d = nc.dram_tensor("bd2p", [128, 128], F32, kind="ExternalInput")
    idf_d = nc.dram_tensor("identf", [128, 128], F32, kind="ExternalInput")
    out_d = nc.dram_tensor("out", [B_LOC, C, H, W], F32, kind="ExternalOutput")

    from contextlib import ExitStack

    with tile.TileContext(nc) as tc, ExitStack() as ctx:
        const = ctx.enter_context(tc.tile_pool(name="const", bufs=1))
        big = ctx.enter_context(tc.tile_pool(name="big", bufs=1))
        xp = ctx.enter_context(tc.tile_pool(name="xp", bufs=2))
        apool = ctx.enter_context(tc.tile_pool(name="ap", bufs=3))
        rpool = ctx.enter_context(tc.tile_pool(name="rp", bufs=3))
        mxp = ctx.enter_context(tc.tile_pool(name="mxp", bufs=2))
        o32p = ctx.enter_context(tc.tile_pool(name="o32", bufs=2))
        misc = ctx.enter_context(tc.tile_pool(name="misc", bufs=1))
        psap = ctx.enter_context(tc.tile_pool(name="psa", bufs=1, space="PSUM"))
        ps1p = ctx.enter_context(tc.tile_pool(name="ps1", bufs=2, space="PSUM"))
        ps2p = ctx.enter_context(tc.tile_pool(name="ps2", bufs=1, space="PSUM"))
        pssp = ctx.enter_context(tc.tile_pool(name="pss", bufs=1, space="PSUM"))

        def load_const(name_d, shape, tag):
            t = const.tile(shape, F32, tag=tag)
            nc.sync.dma_start(out=t, in_=name_d[:, :])
            return t

        BD1f = load_const(bd1_d, [128, 128], "bd1f")
        BD2f = load_const(bd2_d, [128, 128], "bd2f")
        IDTf = load_const(idf_d, [128, 128], "idf")
        W1A = load_const(w1a_d, [128, 16], "w1a")
        W1B = load_const(w1b_d, [128, 16], "w1b")
        W2t = load_const(w2_d, [16, 256], "w2t")
        BD1 = const.tile([128, 128], BF16)
        nc.scalar.copy(out=BD1, in_=BD1f)
        BD2 = const.tile([128, 128], BF16)
        nc.scalar.copy(out=BD2, in_=BD2f)
        IDTb = const.tile([128, 128], BF16)
        nc.scalar.copy(out=IDTb, in_=IDTf)

        # ---- bf16 input via gpsimd SWDGE casting loads (8KB descs) ----
        ip_f = ip_d.rearrange("b (cg cl) h w -> cl b cg (h w)", cg=2)
        out_v = out_d.rearrange("b (cg cl) h w -> cl b cg (h w)", cg=2)
        T16 = []
        for j in range(NJ):
            b, cg = j // 2, j % 2
            t = big.tile([128, 4096], BF16, tag=f"t16_{j}")
            for half in range(2):
                nc.gpsimd.dma_start(
                    out=t[:, 2048 * half : 2048 * (half + 1)],
                    in_=ip_f[:, b, cg, 2048 * half : 2048 * (half + 1)],
                )
            T16.append(t)

        Scols = misc.tile([128, NJ], F32)
        hT = misc.tile([16, 2], F32)
        Zpp = misc.tile([128, NJ], F32)
        scr = pssp.tile([128, 256], F32, tag="scr")

        Xs = [None] * NJ
        Xvs = [None] * NJ
        Mxs = [None] * NJ
        Rs = {}

        def t1(j):
            X = xp.tile([128, 4096], BF16, tag="x")
            for a in range(4):
                psa = psap.tile([128, 1024], BF16, tag="psa")
                for t in range(8):
                    Hc = 8 * a + t
                    nc.tensor.transpose(
                        psa[:, 128 * t : 128 * t + 128],
                        T16[j][:, 128 * Hc : 128 * Hc + 128],
                        IDTb,
                    )
                nc.scalar.copy(out=X[:, 1024 * a : 1024 * (a + 1)], in_=psa)
            Xs[j] = X
            Xvs[j] = X.rearrange("p (hc c) -> p c hc", hc=32)
            Mxs[j] = mxp.tile([128, 128], F32, tag="mx", name=f"mx{j}")

        def s1_mid_q(j, q):
            """s1 matmuls + ACT cast evac + DVE StreamT for one quarter."""
            ps1 = ps1p.tile([128, 1024], F32, tag="ps1")
            for m in range(2):
                nc.tensor.matmul(
                    ps1[:, 512 * m : 512 * m + 512],
                    lhsT=BD1,
                    rhs=Xvs[j][:, 32 * q + 16 * m : 32 * q + 16 * m + 16, :],
                    start=True,
                    stop=True,
                )
            A = apool.tile([128, 1024], BF16, tag="a")
            nc.scalar.copy(out=A, in_=ps1)
            R = rpool.tile([128, 1024], BF16, tag="r")
            nc.vector.transpose(out=R, in_=A)
            Rs[(j, q)] = R

        def s2_red_q(j, q):
            ps2 = ps2p.tile([128, 1024], F32, tag="ps2")
            R = Rs.pop((j, q))
            for m in range(2):
                nc.tensor.matmul(
                    ps2[:, 512 * m : 512 * m + 512],
                    lhsT=BD2,
                    rhs=R[:, 512 * m : 512 * m + 512],
                    start=True,
                    stop=True,
                )
            nc.vector.reduce_max(
                out=Mxs[j][:, 32 * q : 32 * q + 32],
                in_=ps2.rearrange("p (c k) -> p c k", k=32),
                axis=mybir.AxisListType.X,
            )

        def fin(j):
            mxt = scr[:, 0:128]
            nc.tensor.transpose(mxt, Mxs[j], IDTf)
            nc.vector.reduce_max(
                out=Scols[:, j : j + 1], in_=mxt, axis=mybir.AxisListType.X
            )

        def phase_b(b):
            ph = scr[0:16, 128:129]
            nc.tensor.matmul(
                ph, lhsT=W1A, rhs=Scols[:, 2 * b : 2 * b + 1],
                start=True, stop=False,
            )
            nc.tensor.matmul(
                ph, lhsT=W1B, rhs=Scols[:, 2 * b + 1 : 2 * b + 2],
                start=False, stop=True,
            )
            nc.scalar.activation(
                out=hT[:, b : b + 1], in_=ph,
                func=mybir.ActivationFunctionType.Relu,
            )
            pz = scr[:, 132:134]
            for cg in range(2):
                nc.tensor.matmul(
                    pz[:, cg : cg + 1],
                    lhsT=W2t[:, 128 * cg : 128 * cg + 128],
                    rhs=hT[:, b : b + 1],
                    start=True,
                    stop=True,
                )
            nc.scalar.activation(
                out=Zpp[:, 2 * b : 2 * b + 2], in_=pz,
                func=mybir.ActivationFunctionType.Sigmoid,
            )

        def mult_store(j, eng):
            b, cg = j // 2, j % 2
            o = o32p.tile([128, 4096], F32, tag="o32")
            if eng == "gp":
                nc.gpsimd.tensor_tensor(
                    out=o,
                    in0=T16[j],
                    in1=Zpp[:, j : j + 1].broadcast_to([128, 4096]),
                    op=mybir.AluOpType.mult,
                )
            elif eng == "dve":
                nc.vector.tensor_scalar_mul(o, T16[j], Zpp[:, j : j + 1])
            else:
                nc.scalar.mul(out=o, in_=T16[j], mul=Zpp[:, j : j + 1])
            nc.sync.dma_start(out=out_v[:, b, cg, :], in_=o)

        # ---- software-pipelined emission (1-j lookahead) ----
        t1(0)
        for q in range(4):
            s1_mid_q(0, q)
        t1(1)
        for q in range(4):
            s2_red_q(0, q)
            s1_mid_q(1, q)
        fin(0)
        t1(2)
        for q in range(4):
            s2_red_q(1, q)
            s1_mid_q(2, q)
        fin(1)
        phase_b(0)
        mult_store(0, "gp")
        t1(3)
        for q in range(4):
            s2_red_q(2, q)
            s1_mid_q(3, q)
        fin(2)
        mult_store(1, "dve")
        for q in range(4):
            s2_red_q(3, q)
        fin(3)
        phase_b(1)
        mult_store(2, "act")
        mult_store(3, "gp")

    nc.finalize()
    return nc


def get_nc():
    if "nc" not in _NC_CACHE:
        _NC_CACHE["nc"] = _build_nc()
    return _NC_CACHE["nc"]


def make_in_map(ip_shard, w1, w2):
    BD1, BD2p, identf = _constants()
    return {
        "ip": np.ascontiguousarray(ip_shard, dtype=np.float32),
        "w1a": np.ascontiguousarray(w1[0:128], dtype=np.float32),
        "w1b": np.ascontiguousarray(w1[128:256], dtype=np.float32),
        "w2": np.ascontiguousarray(w2, dtype=np.float32),
        "bd1": BD1,
        "bd2p": BD2p,
        "identf": identf,
    }


def kernel(ip, w1, w2):
    assert ip.shape == (B, C, H, W), ip.shape
    nc = get_nc()
    ip = np.ascontiguousarray(ip, dtype=np.float32)
    w1 = np.asarray(w1, dtype=np.float32)
    w2 = np.asarray(w2, dtype=np.float32)
    in_maps = [
        make_in_map(ip[B_LOC * k : B_LOC * (k + 1)], w1, w2)
        for k in range(N_CORES)
    ]
    res = run_bass_kernel_spmd(nc, in_maps, list(range(N_CORES)), **RUN_KWARGS)
    LAST_RESULT.clear()
    LAST_RESULT["exec_time_ns"] = res.exec_time_ns
    LAST_RESULT["profile_json"] = res.profile_json
    return np.concatenate([m["out"] for m in res.results], axis=0)


RUN_KWARGS = {}
LAST_RESULT = {}
